# revision 9
# baseline (speedup 1.0000x reference)
"""Trainium2 Bass kernel for mixed softmax + relu^2 attention (v2).

Reference computation (B=4, S=2048, D=768, H=12, DH=64):
    q = split_heads(hidden @ Wq.T + bq)        # [B,H,S,DH]
    k = split_heads(hidden @ Wk.T + bk)
    v = split_heads(hidden @ Wv.T + bv)
    scores = q @ k.T / sqrt(DH)                # [B,H,S,S]
    attn = m0 * softmax(scores) + m1 * relu(scores)^2,  (m0,m1) = softmax(w_mix)
    out = merge_heads(attn @ v) @ Wo.T + bo

Sharding over 8 NeuronCores: core = (batch b = core//2, head-group g = core%2 of
6 heads).  Each core computes its 6 heads' full SxS attention and a partial
output projection over its 384 context dims; the host sums the two partials
per batch.

Device-side layout ("transposed", k on partitions), per head pair p (2 heads
a0/a1 stacked on partitions 0-63 / 64-127):
  - qk[p] [128, 2S]: Q cols [0,S) (pre-scaled by 1/sqrt(DH) via host-side
    Wq scaling), K cols [S,2S).  Head-major rows.  Evicted from a single
    2-bank PSUM tile with one ACT copy per q-chunk.
  - scoresT tile ss [k=128, 2*512] = K_tile.T @ Q_chunk for both heads
    (row-packed concurrent matmuls via auto tile_position).
  - e = exp(ss) on ACT -> bf16; r = relu(ss)^2 on DVE (custom op) -> bf16.
  - V augmented per head: [alpha*V | beta] where alpha=max(m1,eps),
    beta=alpha/m0; e-AV accumulates [alpha*V|beta].T @ e so row 64 holds
    beta*Z (Z = softmax denominator); r-AV accumulates (alpha*V).T @ r
    col-packed for both heads into one psum tile.
  - combine: ACT evicts pse rows 0-64 -> SBUF; DVE reciprocal of the
    beta*Z rows (PSUM); GpSimd broadcasts 1/(beta*Z), multiplies and adds:
    ctx = ex * zb + xr  (equals m0*V.T e/Z + m1*V.T r by construction).
  - out_partial[o, s] = Wo_part.T @ ctx per 128-row o-tile, interleaved one
    o-tile per k-tile iteration of a later block; shipped fp32; host sums.
"""

from contextlib import ExitStack

import numpy as np
import ml_dtypes

import concourse.bass as bass
import concourse.mybir as mybir
import concourse.tile as tile
from concourse import bacc, dve_ops
from concourse.bass_utils import run_bass_kernel_spmd
from concourse.dve_spec import Spec, Src0, relu as _sp_relu, sq as _sp_sq


def _register_relu_sq():
    """Custom fused DVE op: out = relu(in0)^2 in a single pass."""
    for op in dve_ops.OPS:
        if op.name == "RELU_SQ_ANT":
            return op
    op = dve_ops.DveOp(
        "RELU_SQ_ANT",
        Spec(body=_sp_sq(_sp_relu(Src0)),
             reference=lambda in0: np.maximum(in0, 0.0) ** 2),
        subdim=False,
        uops_sha={"v3": "8abca05ebc329c1b", "v4": "4b83c053374efcdc"},
    )
    dve_ops.OPS.append(op)
    dve_ops.CUSTOM_DVE_SPECS[op.name] = op.spec
    dve_ops._SUB_OPCODE_FOR_NAME[op.name] = (
        dve_ops._CUSTOM_DVE_ROW_BASE + len(dve_ops.OPS) - 1
    )
    return op


RELU_SQ = _register_relu_sq()

B, S, D, H, DH = 4, 2048, 768, 12, 64
NCORES = 8
HL = H // 2          # local heads per core = 6
HPAIRS = HL // 2     # head pairs = 3
DLOC = HL * DH       # local context dims = 384
KTILES = S // 128    # 16
QCHUNK = 512
NQC = S // QCHUNK    # 4
DKT = D // 128       # 6 contraction tiles for projections
OTILES = D // 128    # 6 output-projection row tiles

F32 = mybir.dt.float32
BF16 = mybir.dt.bfloat16
NP_BF16 = ml_dtypes.bfloat16
AF = mybir.ActivationFunctionType
OP = mybir.AluOpType

# AV matmuls consume elementwise results this many k-tiles behind the scores
# matmul, so the in-order PE stream never waits on the elementwise chain.
AV_DELAY = 4
# engine for the outproj psum eviction: "act" | "dve" | "any"
OB_ENGINE = "any"
# engine for the psr psum eviction
XR_ENGINE = "any"
# number of et/rt buffers
EW_BUFS = 7

_KERNEL_CACHE: dict = {}


def build_kernel(m0: float, m1: float, has_bias: bool, repeat: int = 1):
    nc = bacc.Bacc("TRN2", target_bir_lowering=False, debug=False)

    hT = nc.dram_tensor("hT", [D, S], BF16, kind="ExternalInput").ap()
    wqT = nc.dram_tensor("wqT", [D, DLOC], BF16, kind="ExternalInput").ap()
    wkT = nc.dram_tensor("wkT", [D, DLOC], BF16, kind="ExternalInput").ap()
    wvT = nc.dram_tensor("wvT", [D, DLOC], BF16, kind="ExternalInput").ap()
    woT = nc.dram_tensor("woT", [DLOC, D], BF16, kind="ExternalInput").ap()
    if has_bias:
        hb = nc.dram_tensor("hb", [1, S], BF16, kind="ExternalInput").ap()
        wqb = nc.dram_tensor("wqb", [1, DLOC], BF16, kind="ExternalInput").ap()
        wkb = nc.dram_tensor("wkb", [1, DLOC], BF16, kind="ExternalInput").ap()
        wvb = nc.dram_tensor("wvb", [1, DLOC], BF16, kind="ExternalInput").ap()
    out = nc.dram_tensor("out", [D, S], F32, kind="ExternalOutput").ap()

    # branch-mix scales: V columns get alpha, the ones column gets beta so the
    # combine is ctx = ex * (1/(beta*Z)) + xr with no extra scalar ops.
    alpha = max(m1, 1e-18)
    beta = alpha / max(m0, 1e-18)

    with tile.TileContext(nc) as tc, ExitStack() as ctx:
        # ---------------- persistent SBUF ----------------
        pp = ctx.enter_context(tc.tile_pool(name="persist", bufs=1))

        h_t = [pp.tile([128, S], BF16, tag=f"ht{k}", name=f"ht{k}") for k in range(DKT)]
        wq_t = [pp.tile([128, DLOC], BF16, tag=f"wq{k}", name=f"wq{k}") for k in range(DKT)]
        wk_t = [pp.tile([128, DLOC], BF16, tag=f"wk{k}", name=f"wk{k}") for k in range(DKT)]
        wv_t = [pp.tile([128, DLOC], BF16, tag=f"wv{k}", name=f"wv{k}") for k in range(DKT)]
        wo_t = [pp.tile([128, D], BF16, tag=f"wo{c}", name=f"wo{c}") for c in range(HPAIRS)]
        for k in range(DKT):
            nc.sync.dma_start(h_t[k][:], hT[k * 128:(k + 1) * 128, :])
            nc.sync.dma_start(wq_t[k][:], wqT[k * 128:(k + 1) * 128, :])
            nc.sync.dma_start(wk_t[k][:], wkT[k * 128:(k + 1) * 128, :])
            nc.sync.dma_start(wv_t[k][:], wvT[k * 128:(k + 1) * 128, :])
        for c in range(HPAIRS):
            nc.sync.dma_start(wo_t[c][:], woT[c * 128:(c + 1) * 128, :])
        if has_bias:
            hb_t = pp.tile([1, S], BF16, tag="hbt")
            wqb_t = pp.tile([1, DLOC], BF16, tag="wqbt")
            wkb_t = pp.tile([1, DLOC], BF16, tag="wkbt")
            wvb_t = pp.tile([1, DLOC], BF16, tag="wvbt")
            nc.sync.dma_start(hb_t[:], hb[:, :])
            nc.sync.dma_start(wqb_t[:], wqb[:, :])
            nc.sync.dma_start(wkb_t[:], wkb[:, :])
            nc.sync.dma_start(wvb_t[:], wvb[:, :])

        # Q and K side by side so one ACT copy evicts both per q-chunk
        qk_s = [pp.tile([128, 2 * S], BF16, tag=f"qk{p}", name=f"qk{p}") for p in range(HPAIRS)]
        # V with ones column per head (cols 65a..65a+63 = alpha*V, col 65a+64 = beta)
        v1_s = [pp.tile([128, HL * (DH + 1)], BF16, tag=f"v1{t}", name=f"v1{t}") for t in range(KTILES)]
        ctx_s = [pp.tile([128, S], BF16, tag=f"cx{p}", name=f"cx{p}") for p in range(HPAIRS)]

        # ones columns are persistent: written once, never overwritten (the V
        # evictions write a strided AP that skips them)
        for t in range(KTILES):
            v1_3d = v1_s[t][:, :].rearrange("p (a d) -> p a d", d=DH + 1)
            nc.gpsimd.memset(v1_3d[:, :, DH:DH + 1], beta)

        nkt = DKT + (1 if has_bias else 0)

        def ev_engine(name):
            if name == "act":
                return nc.scalar
            if name == "dve":
                return nc.vector
            return nc.any

        def phases(pend_outproj, outpool, obsb, rep):
            # emit one output-projection o-tile: pso accumulate over head
            # pairs, evict, dma
            def outproj_tile(qc, ot):
                cols = bass.ts(qc, QCHUNK)
                pso = outpool.tile([128, QCHUNK], F32, tag="pso",
                                   name=f"pso_r{rep}_{qc}_{ot}")
                orows = bass.ts(ot, 128)
                for c in range(HPAIRS):
                    nc.tensor.matmul(pso[:], wo_t[c][:, orows],
                                     ctx_s[c][:, cols],
                                     start=(c == 0), stop=(c == HPAIRS - 1))
                ob = obsb.tile([128, QCHUNK], F32, tag="ob",
                               name=f"ob_r{rep}_{qc}_{ot}")
                if OB_ENGINE == "act":
                    nc.scalar.activation(ob[:], pso[:], AF.Copy)
                else:
                    ev_engine(OB_ENGINE).tensor_copy(ob[:], pso[:])
                nc.sync.dma_start(out[ot * 128:(ot + 1) * 128, cols], ob[:])

            def drain_outproj(n):
                while len(pend_outproj) > n:
                    qc, ot = pend_outproj.pop(0)
                    outproj_tile(qc, ot)

            # ---------------- phase 1: projections ----------------
            with tc.tile_pool(name="p1ps", bufs=2, space="PSUM") as p1ps, \
                 tc.tile_pool(name="p1v", bufs=2, space="PSUM") as p1vps:
                for p in range(HPAIRS):
                    for qc in range(NQC):
                        # drain any outproj left from the previous rep early in
                        # phase 1 (ctx from the previous rep is long since done)
                        if p == 1 and qc == 0:
                            drain_outproj(0)
                        cols = bass.ts(qc, QCHUNK)
                        psqk = p1ps.tile([128, 2 * QCHUNK], F32, tag="qk")
                        for k in range(nkt):
                            rhs = h_t[k][:, cols] if k < DKT else hb_t[:, cols]
                            st, sp = k == 0, k == nkt - 1
                            wq_l = wq_t[k][:, p * 128:(p + 1) * 128] if k < DKT \
                                else wqb_t[:, p * 128:(p + 1) * 128]
                            wk_l = wk_t[k][:, p * 128:(p + 1) * 128] if k < DKT \
                                else wkb_t[:, p * 128:(p + 1) * 128]
                            nc.tensor.matmul(psqk[:, 0:QCHUNK], wq_l, rhs,
                                             start=st, stop=sp)
                            nc.tensor.matmul(psqk[:, QCHUNK:2 * QCHUNK], wk_l, rhs,
                                             start=st, stop=sp)
                        # one eviction for Q and K: dst is [128, 2, 512] strided
                        qk3 = qk_s[p][:, :].rearrange("p (two s) -> p two s", two=2)
                        ps3 = psqk[:, :].rearrange("p (two s) -> p two s", two=2)
                        nc.scalar.activation(qk3[:, :, qc * QCHUNK:(qc + 1) * QCHUNK],
                                             ps3[:, :, :], AF.Copy)

                for t in range(KTILES):
                    rows = bass.ts(t, 128)
                    psv = p1vps.tile([128, DLOC], F32, tag="v")
                    for k in range(nkt):
                        lhsT = h_t[k][:, rows] if k < DKT else hb_t[:, rows]
                        rhs = wv_t[k][:] if k < DKT else wvb_t[:]
                        nc.tensor.matmul(psv[:], lhsT, rhs, start=(k == 0), stop=(k == nkt - 1))
                    v1_3d = v1_s[t][:, :].rearrange("p (a d) -> p a d", d=DH + 1)
                    psv_3d = psv[:, :].rearrange("p (a d) -> p a d", d=DH)
                    nc.scalar.activation(v1_3d[:, :, 0:DH], psv_3d[:, :, :], AF.Copy,
                                         scale=alpha)

            # ---------------- phase 2: attention ----------------
            with tc.tile_pool(name="scps", bufs=2, space="PSUM") as scps, \
                 tc.tile_pool(name="acps", bufs=1, space="PSUM") as acps, \
                 tc.tile_pool(name="ewsb", bufs=EW_BUFS) as ewsb, \
                 tc.tile_pool(name="cbsb", bufs=2) as cbsb:
                for p in range(HPAIRS):
                    a0, a1 = 2 * p, 2 * p + 1
                    for qc in range(NQC):
                        cols = bass.ts(qc, QCHUNK)
                        pse_a = acps.tile([128, QCHUNK], F32, tag="peA")
                        pse_b = acps.tile([128, QCHUNK], F32, tag="peB")
                        psr = acps.tile([128, QCHUNK], F32, tag="pr")
                        pending = {}

                        def av_mms(t):
                            et, rt = pending.pop(t)
                            st, sp = t == 0, t == KTILES - 1
                            va = v1_s[t][:, a0 * (DH + 1):(a0 + 1) * (DH + 1)]
                            vb = v1_s[t][:, a1 * (DH + 1):(a1 + 1) * (DH + 1)]
                            nc.tensor.matmul(pse_a[0:DH + 1, :], va, et[:, 0:QCHUNK],
                                             start=st, stop=sp)
                            nc.tensor.matmul(pse_b[0:DH + 1, :], vb, et[:, QCHUNK:2 * QCHUNK],
                                             start=st, stop=sp)
                            nc.tensor.matmul(psr[0:64, :], v1_s[t][:, a0 * (DH + 1):a0 * (DH + 1) + DH],
                                             rt[:, 0:QCHUNK], start=st, stop=sp)
                            nc.tensor.matmul(psr[64:128, :], v1_s[t][:, a1 * (DH + 1):a1 * (DH + 1) + DH],
                                             rt[:, QCHUNK:2 * QCHUNK], start=st, stop=sp)

                        for t in range(KTILES):
                            kcols = slice(S + t * 128, S + (t + 1) * 128)
                            qcols = slice(qc * QCHUNK, (qc + 1) * QCHUNK)
                            # both heads' score tiles side by side in one 2-bank
                            # PSUM tile; the two matmuls row-pack (tile_position
                            # (0,0) and (64,0) via base partitions)
                            ss = scps.tile([128, 2 * QCHUNK], F32, tag="s")
                            nc.tensor.matmul(ss[:, 0:QCHUNK], qk_s[p][0:64, kcols],
                                             qk_s[p][0:64, qcols])
                            nc.tensor.matmul(ss[:, QCHUNK:2 * QCHUNK], qk_s[p][64:128, kcols],
                                             qk_s[p][64:128, qcols])

                            et = ewsb.tile([128, 2 * QCHUNK], BF16, tag="e")
                            rt = ewsb.tile([128, 2 * QCHUNK], BF16, tag="r")
                            nc.scalar.activation(et[:], ss[:], AF.Exp)
                            nc.vector._custom_dve(RELU_SQ, out=rt[:], in0=ss[:])
                            pending[t] = (et, rt)
                            if t >= AV_DELAY:
                                av_mms(t - AV_DELAY)
                            # one pending outproj o-tile per mid-block k-tile
                            if 6 <= t < 6 + 2 * OTILES and t % 2 == 0 and pend_outproj:
                                qc_o, ot_o = pend_outproj.pop(0)
                                outproj_tile(qc_o, ot_o)
                        for t in range(KTILES - AV_DELAY, KTILES):
                            av_mms(t)

                        # ---- combine ----
                        # free the accumulator banks promptly: ACT evicts the
                        # context rows and the beta*Z rows (cross-partition
                        # 64->0 copy, proven pattern), then DVE reciprocals
                        # the SBUF zrows.
                        exq = cbsb.tile([128, QCHUNK], F32, tag="exq")
                        xr = cbsb.tile([128, QCHUNK], F32, tag="xr")
                        zrow_a = cbsb.tile([1, QCHUNK], F32, tag="zwa")
                        zrow_b = cbsb.tile([1, QCHUNK], F32, tag="zwb")
                        zrec_a = cbsb.tile([1, QCHUNK], F32, tag="zra")
                        zrec_b = cbsb.tile([1, QCHUNK], F32, tag="zrb")
                        nc.scalar.activation(exq[0:64, :], pse_a[0:64, :], AF.Copy)
                        nc.scalar.activation(zrow_a[0:1, :], pse_a[64:65, :], AF.Copy)
                        nc.scalar.activation(exq[64:128, :], pse_b[0:64, :], AF.Copy)
                        nc.scalar.activation(zrow_b[0:1, :], pse_b[64:65, :], AF.Copy)
                        nc.vector.reciprocal_approx_fast(zrec_a[:], zrow_a[:])
                        nc.vector.reciprocal_approx_fast(zrec_b[:], zrow_b[:])
                        if XR_ENGINE == "act":
                            nc.scalar.activation(xr[:], psr[:], AF.Copy)
                        else:
                            ev_engine(XR_ENGINE).tensor_copy(xr[:], psr[:])
                        # GpSimd: broadcast 1/(beta*Z) and combine
                        zb1 = cbsb.tile([128, QCHUNK], F32, tag="zb1")
                        zb2 = cbsb.tile([128, QCHUNK], F32, tag="zb2")
                        nc.gpsimd.partition_broadcast(zb1[:, :], zrec_a[0:1, :], channels=128)
                        nc.gpsimd.partition_broadcast(zb2[:, :], zrec_b[0:1, :], channels=128)
                        prod = cbsb.tile([128, QCHUNK], F32, tag="prod")
                        nc.gpsimd.tensor_tensor(prod[0:64, :], exq[0:64, :], zb1[0:64, :], op=OP.mult)
                        nc.gpsimd.tensor_tensor(prod[64:128, :], exq[64:128, :], zb2[64:128, :], op=OP.mult)
                        nc.gpsimd.tensor_tensor(ctx_s[p][:, cols], prod[:], xr[:], op=OP.add)

                        # queue this q-chunk's output projection once all head
                        # pairs are done; emitted interleaved in later blocks
                        if p == HPAIRS - 1:
                            for ot in range(OTILES):
                                pend_outproj.append((qc, ot))

        pend_outproj: list = []
        with tc.tile_pool(name="outps", bufs=1, space="PSUM") as outpool, \
             tc.tile_pool(name="obsb", bufs=2) as obsb:
            for _rep in range(repeat):
                phases(pend_outproj, outpool, obsb, _rep)
            # tail: remaining outproj tiles of the last rep
            while pend_outproj:
                qc, ot = pend_outproj.pop(0)
                cols = bass.ts(qc, QCHUNK)
                pso = outpool.tile([128, QCHUNK], F32, tag="pso",
                                   name=f"pso_tail_{qc}_{ot}")
                orows = bass.ts(ot, 128)
                for c in range(HPAIRS):
                    nc.tensor.matmul(pso[:], wo_t[c][:, orows],
                                     ctx_s[c][:, cols],
                                     start=(c == 0), stop=(c == HPAIRS - 1))
                ob = obsb.tile([128, QCHUNK], F32, tag="ob",
                               name=f"ob_tail_{qc}_{ot}")
                nc.scalar.activation(ob[:], pso[:], AF.Copy)
                nc.sync.dma_start(out[ot * 128:(ot + 1) * 128, cols], ob[:])

    nc.compile()
    return nc


def _get_kernel(m0: float, m1: float, has_bias: bool):
    key = (round(m0, 9), round(m1, 9), has_bias)
    if key not in _KERNEL_CACHE:
        _KERNEL_CACHE[key] = build_kernel(m0, m1, has_bias)
    return _KERNEL_CACHE[key]


def make_in_maps(inputs: dict) -> tuple[list[dict], float, float, bool]:
    hidden = np.asarray(inputs["hidden_states"], dtype=np.float32)
    Wq = np.asarray(inputs["Wq"], dtype=np.float32)
    Wk = np.asarray(inputs["Wk"], dtype=np.float32)
    Wv = np.asarray(inputs["Wv"], dtype=np.float32)
    Wo = np.asarray(inputs["Wo"], dtype=np.float32)
    bq = np.asarray(inputs["bq"], dtype=np.float32)
    bk = np.asarray(inputs["bk"], dtype=np.float32)
    bv = np.asarray(inputs["bv"], dtype=np.float32)
    w_mix = np.asarray(inputs["w_mix"], dtype=np.float32)

    e = np.exp(w_mix - w_mix.max())
    mix = e / e.sum()
    m0, m1 = float(mix[0]), float(mix[1])
    has_bias = bool(bq.any() or bk.any() or bv.any())

    qk_scale = 1.0 / float(np.sqrt(DH))

    def bf(x):
        return np.ascontiguousarray(x).astype(NP_BF16)

    in_maps = []
    for core in range(NCORES):
        b, g = core // 2, core % 2
        rows = slice(DLOC * g, DLOC * (g + 1))
        m = {
            "hT": bf(hidden[b].T),
            "wqT": bf(Wq[rows].T * qk_scale),
            "wkT": bf(Wk[rows].T),
            "wvT": bf(Wv[rows].T),
            "woT": bf(Wo[:, rows].T),
        }
        if has_bias:
            m["hb"] = bf(np.ones((1, S), dtype=np.float32))
            m["wqb"] = bf(bq[rows][None, :] * qk_scale)
            m["wkb"] = bf(bk[rows][None, :])
            m["wvb"] = bf(bv[rows][None, :])
        in_maps.append(m)
    return in_maps, m0, m1, has_bias


def assemble_output(results: list[dict], bo: np.ndarray) -> np.ndarray:
    out = np.empty((B, S, D), dtype=np.float32)
    for b in range(B):
        out[b] = (results[2 * b]["out"] + results[2 * b + 1]["out"]).T
    if bo.any():
        out += bo
    return out


def _spot_check(out: np.ndarray, inputs: dict, rng: np.random.Generator) -> bool:
    """Recompute one random query row per batch on the host (covers all 8
    cores' partial outputs) and compare; guards against transient HW faults."""
    hidden = np.asarray(inputs["hidden_states"], dtype=np.float32)
    Wq = np.asarray(inputs["Wq"], dtype=np.float32)
    Wk = np.asarray(inputs["Wk"], dtype=np.float32)
    Wv = np.asarray(inputs["Wv"], dtype=np.float32)
    Wo = np.asarray(inputs["Wo"], dtype=np.float32)
    bq = np.asarray(inputs["bq"], dtype=np.float32)
    bk = np.asarray(inputs["bk"], dtype=np.float32)
    bv = np.asarray(inputs["bv"], dtype=np.float32)
    bo = np.asarray(inputs["bo"], dtype=np.float32)
    w_mix = np.asarray(inputs["w_mix"], dtype=np.float32)
    e = np.exp(w_mix - w_mix.max())
    m0, m1 = e / e.sum()
    for b in range(B):
        s = int(rng.integers(0, S))
        q = (hidden[b, s] @ Wq.T + bq).reshape(H, DH) / np.sqrt(DH)
        k = (hidden[b] @ Wk.T + bk).reshape(S, H, DH)
        v = (hidden[b] @ Wv.T + bv).reshape(S, H, DH)
        scores = np.einsum("hd,khd->hk", q, k)
        sm = np.exp(scores - scores.max(axis=1, keepdims=True))
        sm /= sm.sum(axis=1, keepdims=True)
        attn = m0 * sm + m1 * np.maximum(scores, 0.0) ** 2
        ctx = np.einsum("hk,khd->hd", attn, v).reshape(D)
        want = ctx @ Wo.T + bo
        got = out[b, s]
        rel = np.abs(got - want).max() / max(np.abs(want).max(), 1e-6)
        if not np.isfinite(got).all() or rel > 0.05:
            return False
    return True


def kernel(**inputs) -> np.ndarray:
    in_maps, m0, m1, has_bias = make_in_maps(inputs)
    nc = _get_kernel(m0, m1, has_bias)
    bo = np.asarray(inputs["bo"], dtype=np.float32)
    rng = np.random.default_rng(12345)
    out = None
    for _attempt in range(3):
        res = run_bass_kernel_spmd(nc, in_maps, core_ids=list(range(NCORES)))
        out = assemble_output(res.results, bo)
        if np.isfinite(out).all() and _spot_check(out, inputs, rng):
            return out
    return out
